# revision 6
# baseline (speedup 1.0000x reference)
"""DiffPool layer on 8 trn2 NeuronCores — gather-free dense-adjacency design.

Sharding: 16 block-diagonal graphs -> 2 per core (no cross-core edges).
Per graph (2000 nodes): build dense B[src,dst] (bf16, exact small counts)
on-chip from the edge list via one-hot outer-product matmuls, then all of
agg/deg/tmp become GEMMs against the resident B. h and assign are split
hi+lo bf16 (exact against exact-bf16 B). GIN MLPs run as full-precision
f32 matmuls. Host only shards, packs indices, and assembles the
block-diagonal outputs.
"""
import numpy as np
import ml_dtypes

N = 32000
NPG = 2000
K = 1600
KPG = 100
D = 128
NCORES = 8
GPC = 2           # graphs per core
NT = 128          # node tile
TPG = 16          # node tiles per graph (15*128 + 80)
LAST = NPG - 15 * NT  # 80
NKT = 13          # 1600 = 12*128 + 64
KTS = [128] * 12 + [64]

_CACHE = {}


def _marshal_graph(src_l, dst_l):
    w = src_l >> 7
    nt = dst_l >> 7
    order = np.lexsort((dst_l, nt, w))
    ws, nts = w[order], nt[order]
    sw = (src_l - (w << 7))[order].astype(np.int16)
    dt = (dst_l - (nt << 7))[order].astype(np.int16)
    counts = np.zeros((TPG, TPG), np.int64)
    np.add.at(counts, (w, nt), 1)
    return counts, sw, dt, ws, nts


def _pack_chunks(nch, sw, dt, ws, nts):
    """Place this graph's (bucket-sorted) edges into the shared padded
    chunk layout. nch[w,nt] = chunks per bucket (shared across cores)."""
    total = int(nch.sum()) * 128
    srcw = np.full(total, 255, np.int16)
    dstt = np.full(total, 255, np.int16)
    flat = nch.reshape(-1)
    starts = np.zeros(TPG * TPG, np.int64)
    starts[1:] = np.cumsum(flat * 128)[:-1]
    key = ws.astype(np.int64) * TPG + nts
    bcounts = np.bincount(key, minlength=TPG * TPG)
    ofs = 0
    for b in range(TPG * TPG):
        c = int(bcounts[b])
        if c:
            srcw[starts[b]:starts[b] + c] = sw[ofs:ofs + c]
            dstt[starts[b]:starts[b] + c] = dt[ofs:ofs + c]
            ofs += c
    return srcw.reshape(-1, 128), dstt.reshape(-1, 128)


def _chunk_meta(nch):
    """Chunk records (w, ng, nt, start, stop); PSUM group = (w, ng) where
    ng = nt//4 covers one 2KB bank [128, 512]."""
    meta = []
    for w in range(TPG):
        for ng in range(4):
            group = []
            for j in range(4):
                nt = ng * 4 + j
                for _ in range(int(nch[w, nt])):
                    group.append(nt)
            assert group, f"empty psum group w={w} ng={ng}"
            for i, nt in enumerate(group):
                meta.append((w, ng, nt, i == 0, i == len(group) - 1))
    return meta


def _build(nch_by_slot):
    import concourse.bacc as bacc
    import concourse.tile as tile
    from concourse import mybir

    f32 = mybir.dt.float32
    bf16 = mybir.dt.bfloat16
    i16 = mybir.dt.int16
    AL = mybir.AluOpType
    AF = mybir.ActivationFunctionType

    nc = bacc.Bacc()
    NCH = [int(nch.sum()) for nch in nch_by_slot]
    meta = [_chunk_meta(nch_by_slot[s]) for s in range(GPC)]

    def I(name, shape, dt):
        return nc.dram_tensor(name, shape, dt, kind="ExternalInput").ap()

    hhi_d = I("hhi", [GPC, NPG, D], bf16)
    hlo_d = I("hlo", [GPC, NPG, D], bf16)
    hT_d = I("hT", [GPC, D, NPG], f32)
    srcw_d = [I(f"srcw{s}", [128, NCH[s]], i16) for s in range(GPC)]
    dstt_d = [I(f"dstt{s}", [128, NCH[s]], i16) for s in range(GPC)]
    iota8_d = I("iota8", [128, 8 * 128], i16)
    w1f_d = I("w1f", [D, D], f32)
    b1f_d = I("b1f", [D, 1], f32)
    w2f_d = I("w2f", [D, D], f32)
    b2fb_d = I("b2fb", [128, D], f32)
    w1p_d = I("w1p", [D, K], f32)
    b1p_d = I("b1p", [128, NKT], f32)
    w2p_d = I("w2p", [GPC, 128, NKT * KPG], f32)   # ktile-major packed
    b2pb_d = I("b2pb", [GPC, 128, KPG], f32)
    hp_d = nc.dram_tensor("hp", [GPC, KPG, D], f32, kind="ExternalOutput").ap()
    adj_d = nc.dram_tensor("adj", [GPC, KPG, KPG], f32, kind="ExternalOutput").ap()

    with tile.TileContext(nc) as tc:
        with (
            tc.tile_pool(name="const", bufs=1) as cp,
            tc.tile_pool(name="bslab", bufs=1) as bp,
            tc.tile_pool(name="oneh", bufs=2) as ohp,
            tc.tile_pool(name="hres", bufs=1) as hp_,
            tc.tile_pool(name="mlp", bufs=1) as mp,
            tc.tile_pool(name="small", bufs=2) as sp,
            tc.tile_pool(name="ps512", bufs=3, space="PSUM") as p512,
            tc.tile_pool(name="ps128", bufs=3, space="PSUM") as p128,
            tc.tile_pool(name="psdeg", bufs=1, space="PSUM") as pdg,
            tc.tile_pool(name="psacc", bufs=1, space="PSUM") as pac,
        ):
            iota8 = cp.tile([128, 8 * 128], i16)
            nc.sync.dma_start(out=iota8[:], in_=iota8_d[:])
            w1f = cp.tile([D, D], f32); nc.sync.dma_start(out=w1f[:], in_=w1f_d[:])
            b1f = cp.tile([D, 1], f32); nc.sync.dma_start(out=b1f[:], in_=b1f_d[:])
            w2f = cp.tile([D, D], f32); nc.sync.dma_start(out=w2f[:], in_=w2f_d[:])
            b2fb = cp.tile([128, D], f32); nc.sync.dma_start(out=b2fb[:], in_=b2fb_d[:])
            w1p = cp.tile([D, K], f32); nc.sync.dma_start(out=w1p[:], in_=w1p_d[:])
            b1p = cp.tile([128, NKT], f32); nc.sync.dma_start(out=b1p[:], in_=b1p_d[:])
            ones_bf = cp.tile([128, 1], bf16); nc.vector.memset(ones_bf[:], 1.0)
            onesrow = cp.tile([1, 128], f32); nc.vector.memset(onesrow[:], 1.0)

            for s in range(GPC):
                srcw = cp.tile([128, NCH[s]], i16, tag="srcw")
                nc.sync.dma_start(out=srcw[:], in_=srcw_d[s][:])
                dstt = cp.tile([128, NCH[s]], i16, tag="dstt")
                nc.sync.dma_start(out=dstt[:], in_=dstt_d[s][:])
                w2p = cp.tile([128, NKT * KPG], f32, tag="w2p")
                nc.sync.dma_start(out=w2p[:], in_=w2p_d[s][:])
                b2pb = cp.tile([128, KPG], f32, tag="b2pb")
                nc.sync.dma_start(out=b2pb[:], in_=b2pb_d[s][:])
                hhi, hlo = [], []
                for w in range(TPG):
                    kk = NT if w < 15 else LAST
                    t1 = hp_.tile([128, D], bf16, tag=f"hhi{w}")
                    nc.sync.dma_start(out=t1[0:kk, :], in_=hhi_d[s, w * NT:w * NT + kk, :])
                    t2 = hp_.tile([128, D], bf16, tag=f"hlo{w}")
                    nc.sync.dma_start(out=t2[0:kk, :], in_=hlo_d[s, w * NT:w * NT + kk, :])
                    hhi.append(t1); hlo.append(t2)
                hT = hp_.tile([D, NPG], f32, tag="hT")
                nc.sync.dma_start(out=hT[:], in_=hT_d[s][:])

                # ---- one-hots (batched) + B slabs ----
                Bw = [bp.tile([128, TPG * NT], bf16, tag=f"B{w}", name=f"B{w}") for w in range(TPG)]
                m = meta[s]
                smats, rmats = [], []
                for b0 in range(0, len(m), 8):
                    nb = min(8, len(m) - b0)
                    S8 = ohp.tile([128, 8 * 128], bf16, tag="S8")
                    R8 = ohp.tile([128, 8 * 128], bf16, tag="R8")
                    dsl = dstt[:, b0:b0 + nb].rearrange("p (c o) -> p c o", o=1)\
                        .broadcast_to([128, nb, 128])
                    ssl = srcw[:, b0:b0 + nb].rearrange("p (c o) -> p c o", o=1)\
                        .broadcast_to([128, nb, 128])
                    io = iota8[:, 0:nb * 128].rearrange("p (c f) -> p c f", f=128)
                    nc.vector.tensor_tensor(
                        S8[:, 0:nb * 128].rearrange("p (c f) -> p c f", f=128),
                        io, dsl, AL.is_equal)
                    nc.vector.tensor_tensor(
                        R8[:, 0:nb * 128].rearrange("p (c f) -> p c f", f=128),
                        io, ssl, AL.is_equal)
                    for j in range(nb):
                        smats.append(S8[:, j * 128:(j + 1) * 128])
                        rmats.append(R8[:, j * 128:(j + 1) * 128])
                pg = None
                for ci, (w, ng, nt, st, sp_) in enumerate(m):
                    if st:
                        pg = p512.tile([128, 512], f32, tag="T512")
                    nc.tensor.matmul(pg[:, (nt % 4) * 128:(nt % 4) * 128 + 128],
                                     lhsT=rmats[ci], rhs=smats[ci],
                                     start=st, stop=sp_, skip_group_check=True)
                    if sp_:
                        nc.scalar.copy(Bw[w][:, ng * 512:ng * 512 + 512], pg[:])

                # ---- agg / deg / x^T ----
                xT = hp_.tile([D, NPG], f32, tag="xT")
                for q in range(4):
                    c0 = q * 512
                    cw = 512 if q < 3 else NPG - 1536
                    pa = p512.tile([128, 512], f32, tag="T512")
                    pd = pdg.tile([1, 512], f32, tag="deg")
                    for w in range(TPG):
                        kk = NT if w < 15 else LAST
                        nc.tensor.matmul(pa[:, 0:cw], lhsT=hhi[w][0:kk, :],
                                         rhs=Bw[w][0:kk, c0:c0 + cw],
                                         start=(w == 0), stop=False)
                        nc.tensor.matmul(pa[:, 0:cw], lhsT=hlo[w][0:kk, :],
                                         rhs=Bw[w][0:kk, c0:c0 + cw],
                                         start=False, stop=(w == 15))
                        nc.tensor.matmul(pd[:, 0:cw], lhsT=ones_bf[0:kk, :],
                                         rhs=Bw[w][0:kk, c0:c0 + cw],
                                         start=(w == 0), stop=(w == 15))
                    degrow = sp.tile([1, 512], f32, tag="degrow")
                    nc.scalar.copy(degrow[:, 0:cw], pd[:, 0:cw])
                    nc.vector.tensor_scalar_max(degrow[:, 0:cw], degrow[:, 0:cw], 1.0)
                    rcp = sp.tile([1, 512], f32, tag="rcp")
                    nc.vector.reciprocal(rcp[:, 0:cw], degrow[:, 0:cw])
                    prb = p512.tile([128, 512], f32, tag="T512")
                    nc.tensor.matmul(prb[:, 0:cw], lhsT=onesrow[0:1, :],
                                     rhs=rcp[0:1, 0:cw], start=True, stop=True)
                    rcpb = sp.tile([128, 512], f32, tag="rcpb")
                    nc.scalar.copy(rcpb[:, 0:cw], prb[:, 0:cw])
                    aggs = sp.tile([128, 512], f32, tag="aggs")
                    nc.scalar.copy(aggs[:, 0:cw], pa[:, 0:cw])
                    nc.vector.tensor_tensor(aggs[:, 0:cw], aggs[:, 0:cw],
                                            rcpb[:, 0:cw], AL.mult)
                    nc.vector.tensor_tensor(xT[:, c0:c0 + cw], aggs[:, 0:cw],
                                            hT[:, c0:c0 + cw], AL.add)

                # ---- MLP blocks ----
                assign_f, assign_hi, assign_lo, feats = [], [], [], []
                php = pac.tile([KPG, D], f32, tag="acc")
                for blk in range(4):
                    c0 = blk * 512
                    cw = 512 if blk < 3 else NPG - 1536
                    hid = []
                    for kt in range(NKT):
                        mk = KTS[kt]
                        ph = p512.tile([128, 512], f32, tag="T512")
                        nc.tensor.matmul(ph[0:mk, 0:cw],
                                         lhsT=w1p[:, kt * 128:kt * 128 + mk],
                                         rhs=xT[:, c0:c0 + cw], start=True, stop=True)
                        ht = mp.tile([128, 512], f32, tag=f"hid{kt}")
                        nc.scalar.activation(ht[0:mk, 0:cw], ph[0:mk, 0:cw], AF.Relu,
                                             bias=b1p[0:mk, kt:kt + 1])
                        hid.append(ht)
                    pf1 = p512.tile([128, 512], f32, tag="T512")
                    nc.tensor.matmul(pf1[:, 0:cw], lhsT=w1f[:], rhs=xT[:, c0:c0 + cw],
                                     start=True, stop=True)
                    f1 = mp.tile([128, 512], f32, tag="f1")
                    nc.scalar.activation(f1[:, 0:cw], pf1[:, 0:cw], AF.Relu, bias=b1f[:])
                    for j in range(4):
                        nt = blk * 4 + j
                        mm = NT if nt < 15 else LAST
                        pa2 = p128.tile([128, 128], f32, tag="T128")
                        for kt in range(NKT):
                            mk = KTS[kt]
                            nc.tensor.matmul(pa2[0:mm, 0:KPG],
                                             lhsT=hid[kt][0:mk, j * 128:j * 128 + mm],
                                             rhs=w2p[0:mk, kt * KPG:(kt + 1) * KPG],
                                             start=(kt == 0), stop=(kt == NKT - 1))
                        zs = sp.tile([128, KPG], f32, tag="zs")
                        nc.vector.tensor_tensor(zs[0:mm, :], pa2[0:mm, 0:KPG],
                                                b2pb[0:mm, :], AL.add)
                        zr = sp.tile([128, KPG], f32, tag="zr")
                        nc.scalar.activation(zr[0:mm, :], zs[0:mm, :], AF.Relu)
                        mx = sp.tile([128, 1], f32, tag="mx")
                        nc.vector.tensor_reduce(mx[0:mm, :], zr[0:mm, :],
                                                axis=mybir.AxisListType.X, op=AL.max)
                        ngm = sp.tile([128, 1], f32, tag="ngm")
                        nc.vector.tensor_scalar_mul(ngm[0:mm, :], mx[0:mm, :], -1.0)
                        es = sp.tile([128, KPG], f32, tag="es")
                        ssum = sp.tile([128, 1], f32, tag="ssum")
                        nc.scalar.activation(es[0:mm, :], zr[0:mm, :], AF.Exp,
                                             bias=ngm[0:mm, :], accum_out=ssum[0:mm, :])
                        rs2 = sp.tile([128, 1], f32, tag="rs2")
                        nc.vector.reciprocal(rs2[0:mm, :], ssum[0:mm, :])
                        af = hp_.tile([128, KPG], f32, tag=f"af{nt}")
                        nc.vector.tensor_scalar_mul(af[0:mm, :], es[0:mm, :],
                                                    rs2[0:mm, :])
                        ah = hp_.tile([128, KPG], bf16, tag=f"ah{nt}")
                        nc.vector.tensor_copy(ah[0:mm, :], af[0:mm, :])
                        al = hp_.tile([128, KPG], bf16, tag=f"al{nt}")
                        nc.vector.tensor_tensor(al[0:mm, :], af[0:mm, :],
                                                ah[0:mm, :], AL.subtract)
                        assign_f.append(af); assign_hi.append(ah); assign_lo.append(al)
                        pf2 = p128.tile([128, 128], f32, tag="T128")
                        nc.tensor.matmul(pf2[0:mm, :],
                                         lhsT=f1[:, j * 128:j * 128 + mm],
                                         rhs=w2f[:], start=True, stop=True)
                        fb = sp.tile([128, D], f32, tag="fb")
                        nc.vector.tensor_tensor(fb[0:mm, :], pf2[0:mm, 0:D],
                                                b2fb[0:mm, :], AL.add)
                        fo = hp_.tile([128, D], f32, tag=f"feat{nt}")
                        nc.scalar.activation(fo[0:mm, :], fb[0:mm, :], AF.Relu)
                        feats.append(fo)
                        nc.tensor.matmul(php[:, :], lhsT=af[0:mm, :], rhs=fo[0:mm, :],
                                         start=(nt == 0), stop=(nt == 15))
                hps = sp.tile([KPG, D], f32, tag="hps")
                nc.scalar.copy(hps[:], php[:])
                nc.sync.dma_start(out=hp_d[s], in_=hps[:])

                # ---- tmp + adj ----
                padj = pac.tile([KPG, KPG], f32, tag="acc")
                for nt in range(TPG):
                    mm = NT if nt < 15 else LAST
                    pt = p128.tile([128, 128], f32, tag="T128")
                    for w in range(TPG):
                        kk = NT if w < 15 else LAST
                        nc.tensor.matmul(pt[0:mm, 0:KPG],
                                         lhsT=Bw[w][0:kk, nt * 128:nt * 128 + mm],
                                         rhs=assign_hi[w][0:kk, :],
                                         start=(w == 0), stop=False)
                        nc.tensor.matmul(pt[0:mm, 0:KPG],
                                         lhsT=Bw[w][0:kk, nt * 128:nt * 128 + mm],
                                         rhs=assign_lo[w][0:kk, :],
                                         start=False, stop=(w == 15))
                    tn = sp.tile([128, KPG], f32, tag="tn")
                    nc.scalar.copy(tn[0:mm, :], pt[0:mm, 0:KPG])
                    nc.tensor.matmul(padj[:, :], lhsT=assign_f[nt][0:mm, :],
                                     rhs=tn[0:mm, :], start=(nt == 0), stop=(nt == 15))
                adjs = sp.tile([KPG, KPG], f32, tag="adjs")
                nc.scalar.copy(adjs[:], padj[:])
                nc.sync.dma_start(out=adj_d[s], in_=adjs[:])

                assign_f.clear(); assign_hi.clear(); assign_lo.clear(); feats.clear()

    nc.compile()
    return nc


def _prep(h, src, dst):
    packed = []
    gid = dst.astype(np.int64) // NPG
    for c in range(NCORES):
        slots = []
        for s in range(GPC):
            g = c * GPC + s
            msk = gid == g
            slots.append(_marshal_graph((src[msk].astype(np.int64) - g * NPG),
                                        (dst[msk].astype(np.int64) - g * NPG)))
        packed.append(slots)
    nch_by_slot = []
    for s in range(GPC):
        cmax = np.maximum.reduce([packed[c][s][0] for c in range(NCORES)])
        nch_by_slot.append((cmax + 127) // 128)
    return packed, nch_by_slot


def kernel(h, src, dst, W1f, b1f, W2f, b2f, W1p, b1p, W2p, b2p):
    from concourse.bass_utils import run_bass_kernel_spmd

    h = np.asarray(h, np.float32)
    src = np.asarray(src); dst = np.asarray(dst)
    packed, nch_by_slot = _prep(h, src, dst)

    key = tuple(tuple(int(x) for x in nch.reshape(-1)) for nch in nch_by_slot)
    if key not in _CACHE:
        _CACHE[key] = _build(nch_by_slot)
    nc = _CACHE[key]

    iota8 = np.tile(np.arange(128, dtype=np.int16), (128, 8))
    b1p_pack = np.pad(np.asarray(b1p, np.float32), (0, NKT * 128 - K)).reshape(NKT, 128).T.copy()
    in_maps = []
    for c in range(NCORES):
        im = {
            "iota8": iota8,
            "w1f": np.ascontiguousarray(W1f, np.float32),
            "b1f": np.asarray(b1f, np.float32).reshape(D, 1),
            "w2f": np.ascontiguousarray(W2f, np.float32),
            "b2fb": np.tile(np.asarray(b2f, np.float32).reshape(1, D), (128, 1)),
            "w1p": np.ascontiguousarray(W1p, np.float32),
            "b1p": b1p_pack,
        }
        hhi_s, hlo_s, hT_s, w2p_s, b2pb_s = [], [], [], [], []
        for s in range(GPC):
            g = c * GPC + s
            hg = h[g * NPG:(g + 1) * NPG]
            hhi = hg.astype(ml_dtypes.bfloat16)
            hlo = (hg - hhi.astype(np.float32)).astype(ml_dtypes.bfloat16)
            hhi_s.append(hhi); hlo_s.append(hlo)
            hT_s.append(np.ascontiguousarray(hg.T))
            wp = np.asarray(W2p[:, g * KPG:(g + 1) * KPG], np.float32)
            wp = np.pad(wp, ((0, NKT * 128 - K), (0, 0)))
            w2p_s.append(wp.reshape(NKT, 128, KPG).transpose(1, 0, 2).reshape(128, NKT * KPG).copy())
            b2pb_s.append(np.tile(np.asarray(b2p[g * KPG:(g + 1) * KPG], np.float32).reshape(1, KPG), (128, 1)))
            _, sw, dt_, ws, nts = packed[c][s]
            srcw, dstt = _pack_chunks(nch_by_slot[s], sw, dt_, ws, nts)
            im[f"srcw{s}"] = np.ascontiguousarray(srcw.T)
            im[f"dstt{s}"] = np.ascontiguousarray(dstt.T)
        im["hhi"] = np.stack(hhi_s); im["hlo"] = np.stack(hlo_s)
        im["hT"] = np.stack(hT_s).astype(np.float32)
        im["w2p"] = np.stack(w2p_s); im["b2pb"] = np.stack(b2pb_s)
        in_maps.append(im)

    res = run_bass_kernel_spmd(nc, in_maps, list(range(NCORES)))

    h_pool = np.zeros((K, D), np.float32)
    adj_new = np.zeros((K, K), np.float32)
    for c in range(NCORES):
        for s in range(GPC):
            g = c * GPC + s
            h_pool[g * KPG:(g + 1) * KPG] = res.results[c]["hp"][s]
            adj_new[g * KPG:(g + 1) * KPG, g * KPG:(g + 1) * KPG] = res.results[c]["adj"][s]
    return adj_new, h_pool
